# revision 30
# baseline (speedup 1.0000x reference)
"""Trainium2 Bass kernel for LocalPatternFilter.

Reference computation (per (b, h, c) row of length K=1024):
  1. gather window  g = X[b, c, pos[h] : pos[h]+K]
  2. fmax = max|g|;  w = g * hann / fmax
  3. acov = ifftshift(irfft(|rfft(w)|^2))   (= circular autocorrelation)

Implemented as dense DFT matmuls on the tensor engine:
  - X converted to bf16 on host; windows gathered bf16 (indirect DMA).
  - rfft via fp8(e4m3) DoubleRow matmuls (contraction 256/instr, 2 fp8
    weights per PE cell): cos/sin matrices with hann folded in, quantized
    to fp8; window data cast bf16->fp8 during the PSUM->SBUF copy after
    the (bf16) PE transposes.
  - power spectrum P = Re^2 + Im^2 in bf16 (squares split across the
    Scalar and Vector engines).
  - irfft + ifftshift + output symmetry via a bf16 (640 x 520) cos matrix;
    1/fmax^2 folded into the PSUM->SBUF scale copies (Scalar fwd half,
    Vector reversed half). Output y in bf16, upcast to f32 on host.

Sharding: data-parallel over batch, 2 batches per core on 8 cores.
"""

import json

import ml_dtypes
import numpy as np

import concourse.bass as bass
import concourse.bass2jax as bass2jax
import concourse.bass_utils as bass_utils
import concourse.tile as tile
from concourse import mybir
from concourse.bass_utils import run_bass_kernel_spmd

# ---------------------------------------------------------------------------
# The walrus build in this container accepts at most ONE sync-wait command per
# TPB instruction ("Too many sync wait commands" in setupSyncWait), while Tile
# emits several (multi-queue DMA deps, the kernel-tail drain). Legalize the
# serialized BIR before compiling: hoist excess waits onto preceding
# same-engine wait-only EventSemaphore instructions. Engines execute their
# instruction streams in order, so this is semantically identical.
# ---------------------------------------------------------------------------
_MAX_WAITS = 1


def _legalize_bir(bir_bytes):
    m = json.loads(bir_bytes)
    counter = [0]

    def fix_block(blk):
        insts = blk.get("instructions")
        if not isinstance(insts, list):
            return
        out = []
        for inst in insts:
            si = inst.get("sync_info") or {}
            waits = si.get("on_wait") or []
            if isinstance(inst.get("opcode"), str) and len(waits) > _MAX_WAITS:
                keep = waits[-_MAX_WAITS:]
                for w in waits[:-_MAX_WAITS]:
                    counter[0] += 1
                    out.append(
                        {
                            "debug": inst.get("debug", 0),
                            "engine": inst["engine"],
                            "ins": [],
                            "name": f"LGW-{counter[0]}-{inst['name']}",
                            "opcode": "EventSemaphore",
                            "outs": [],
                            "sync_info": {"on_update": [], "on_wait": [w]},
                        }
                    )
                si = dict(si)
                si["on_wait"] = keep
                inst = dict(inst)
                inst["sync_info"] = si
            out.append(inst)
        blk["instructions"] = out

    def walk(o):
        if isinstance(o, dict):
            if "instructions" in o:
                fix_block(o)
            for v in o.values():
                walk(v)
        elif isinstance(o, list):
            for v in o:
                walk(v)

    walk(m)
    return json.dumps(m).encode()


_orig_compile_bir_kernel = bass_utils.compile_bir_kernel


def _legalizing_compile_bir_kernel(bir_json, *args, **kwargs):
    if isinstance(bir_json, str):
        bir_json = bir_json.encode()
    return _orig_compile_bir_kernel(_legalize_bir(bir_json), *args, **kwargs)


if bass_utils.compile_bir_kernel is not _legalizing_compile_bir_kernel:
    bass_utils.compile_bir_kernel = _legalizing_compile_bir_kernel
    bass2jax.compile_bir_kernel = _legalizing_compile_bir_kernel

# positions = int32(jnp.linspace(0, L-2K-1, H)) for L=131072, K=1024, H=128
# (verified identical to the jax reference's values)
POS = [
    0, 1015, 2031, 3047, 4063, 5079, 6095, 7111, 8127, 9143, 10159, 11175,
    12191, 13207, 14223, 15238, 16254, 17270, 18286, 19302, 20318, 21334,
    22350, 23366, 24382, 25398, 26414, 27430, 28446, 29461, 30477, 31493,
    32509, 33525, 34541, 35557, 36573, 37589, 38605, 39621, 40637, 41653,
    42669, 43684, 44700, 45716, 46732, 47748, 48764, 49780, 50796, 51812,
    52828, 53844, 54860, 55876, 56892, 57907, 58923, 59939, 60955, 61971,
    62987, 64003, 65019, 66035, 67051, 68067, 69083, 70099, 71115, 72130,
    73146, 74162, 75178, 76194, 77210, 78226, 79242, 80258, 81274, 82290,
    83306, 84322, 85338, 86353, 87369, 88385, 89401, 90417, 91433, 92449,
    93465, 94481, 95497, 96513, 97529, 98545, 99561, 100576, 101592, 102608,
    103624, 104640, 105656, 106672, 107688, 108704, 109720, 110736, 111752,
    112768, 113784, 114799, 115815, 116831, 117847, 118863, 119879, 120895,
    121911, 122927, 123943, 124959, 125975, 126991, 128007, 129023,
]

N_CORES = 8
B_FULL, C_DIM, L_DIM = 16, 8, 131072
K_DIM, H_DIM = 1024, 128
B_LOC = B_FULL // N_CORES          # batches per core
M_PAD = 640                        # rfft bins 0..512, padded to 5*128
CS_W = 1152                        # 640 windowed-cos cols + 512 windowed-sin
BF16 = mybir.dt.bfloat16
FP8 = mybir.dt.float8e4
F32 = mybir.dt.float32

NP_BF16 = ml_dtypes.bfloat16
NP_FP8 = ml_dtypes.float8_e4m3

_prog_cache = {}


def make_constants():
    K = K_DIM
    k = np.arange(K)[:, None].astype(np.float64)
    hann = 0.5 * (1.0 - np.cos(2.0 * np.pi * np.arange(K) / K))[:, None]
    m = np.arange(M_PAD)[None, :].astype(np.float64)
    cs_cos = hann * np.cos(2 * np.pi * k * m / K)
    cs_cos[:, 513:] = 0.0
    m2 = np.arange(512)[None, :].astype(np.float64)
    cs_sin = hann * np.sin(2 * np.pi * k * m2 / K)
    # cs1[t, p, :] = [win*cos m=0..639 | win*sin m=0..511] for k = 128 t + p
    cs1 = np.concatenate([cs_cos, cs_sin], axis=1)  # (1024, 1152)
    cs1 = cs1.reshape(8, 128, CS_W).astype(NP_FP8)

    # irfft weights for acov[d], d = 0..512 only; the full output row is
    # y[512+d] = acov[d], y[j] = acov[512-j] (acov is even), written via a
    # reversed-stride copy.
    wt = np.ones(M_PAD)
    wt[1:512] = 2.0
    wt[512] = 1.0
    wt[513:] = 0.0
    mm = np.arange(M_PAD)[:, None].astype(np.float64)
    d = np.arange(513)[None, :].astype(np.float64)
    c2 = wt[:, None] * np.cos(2 * np.pi * mm * d / K) / K
    c2 = np.concatenate([c2, np.zeros((M_PAD, 7))], axis=1)  # pad to 520
    c2 = c2.reshape(5, 128, 520).astype(NP_BF16)

    # gather start offsets (elements into flat [B_LOC*C*L] x) per tile row:
    # tile j = g*4 + i; row p = hh*8 + c; h = (g%2)*64 + 16*i + hh
    gidx = np.zeros((16, 128), dtype=np.int32)
    for g in range(2 * B_LOC):
        b = g // 2
        h0 = (g % 2) * 64
        for i in range(4):
            for hh in range(16):
                h = h0 + 16 * i + hh
                for c in range(C_DIM):
                    gidx[g * 4 + i, hh * 8 + c] = (
                        b * C_DIM * L_DIM + c * L_DIM + POS[h]
                    )
    return {"cs1": cs1, "c2": c2, "gidx": gidx}


def build_program():
    nc = bass.Bass("TRN2", target_bir_lowering=False, debug=False,
                   num_swdge_queues=4)
    x = nc.dram_tensor("x", [B_LOC, C_DIM, L_DIM], BF16, kind="ExternalInput").ap()
    cs1 = nc.dram_tensor("cs1", [8, 128, CS_W], FP8, kind="ExternalInput").ap()
    c2 = nc.dram_tensor("c2", [5, 128, 520], BF16, kind="ExternalInput").ap()
    gidx = nc.dram_tensor("gidx", [16, 128], mybir.dt.int32, kind="ExternalInput").ap()
    y = nc.dram_tensor(
        "y", [B_LOC, H_DIM, C_DIM, K_DIM], BF16, kind="ExternalOutput"
    ).ap()

    with tile.TileContext(nc) as tc:
        with (
            tc.tile_pool(name="singles", bufs=1) as singles,
            tc.tile_pool(name="gather", bufs=8) as gpool,
            tc.tile_pool(name="wt", bufs=2) as wtpool,
            tc.tile_pool(name="pp", bufs=2) as ppool,
            tc.tile_pool(name="yy", bufs=4) as ypool,
            tc.tile_pool(name="small", bufs=16) as smallpool,
            tc.tile_pool(name="sq", bufs=3) as sqpool,
            tc.tile_pool(name="mm1_ps", bufs=2, space="PSUM") as mm1_ps_pool,
            tc.tile_pool(name="mm2_ps", bufs=2, space="PSUM") as mm2_ps_pool,
        ):
            # gidx first so gathers can start immediately; DFT matrices
            # stream in as single batched DMAs behind it
            gidx_sb = singles.tile([128, 16], mybir.dt.int32)
            nc.sync.dma_start(out=gidx_sb, in_=gidx.rearrange("t p -> p t"))
            x_flat = x.rearrange("b c l -> (b c) l")
            cs1_sb = singles.tile([128, 8, CS_W], FP8)
            nc.sync.dma_start(out=cs1_sb, in_=cs1.rearrange("t p m -> p t m"))
            c2_sb = singles.tile([128, 5, 520], BF16)
            nc.sync.dma_start(out=c2_sb, in_=c2.rearrange("t p n -> p t n"))

            # HAM warmup: the PE clock-gate needs ~3.4us of sustained matmul
            # activity to unthrottle 1.2 -> 2.4 GHz. Burn junk DoubleRow
            # matmuls on cs1 while the first gathers are still in flight so
            # the real pipeline starts at full clock.
            warm = mm1_ps_pool.tile([128, 1024], F32, tag="mm1")
            for w in range(14):
                nc.tensor.matmul(
                    warm[:, 0:512],
                    cs1_sb[:, 0:2, 0:128],
                    cs1_sb[:, 0:2, 0:512],
                    start=(w == 0),
                    stop=(w == 13),
                    perf_mode=mybir.MatmulPerfMode.DoubleRow,
                )

            # 4 groups of 512 rows; row = b*1024 + h*8 + c
            for g in range(2 * B_LOC):
                b = g // 2
                h0 = (g % 2) * 64
                gts = []
                inv2s = []
                for i in range(4):
                    gt = gpool.tile([128, K_DIM], BF16, tag="gt")
                    j = g * 4 + i
                    gd = nc.gpsimd.indirect_dma_start(
                        out=gt[:],
                        out_offset=None,
                        in_=x_flat,
                        in_offset=bass.IndirectOffsetOnAxis(
                            ap=gidx_sb[:, j : j + 1], axis=1
                        ),
                    )
                    qi = j % 4
                    if qi:
                        gd.ins.queue = f"qPoolDynamic{qi}"  # spread SWDGE queues
                    fm = smallpool.tile([128, 1], BF16, tag="fm")
                    nc.vector.reduce_max(
                        out=fm, in_=gt,
                        axis=mybir.AxisListType.X,
                        apply_absolute_value=True,
                    )
                    inv = smallpool.tile([128, 1], F32, tag="inv")
                    nc.vector.reciprocal(out=inv, in_=fm)
                    inv2 = smallpool.tile([128, 1], F32, tag="inv2")
                    nc.vector.tensor_mul(inv2, inv, inv)
                    gts.append(gt)
                    inv2s.append(inv2)

                # transpose gathered rows (bf16) to [k, row] layout with the
                # DMA xbar (no PE time, no PSUM), then one SWDGE cast-DMA
                # converts the whole group's windows bf16 -> fp8 for the
                # DoubleRow matmuls.
                wtb = wtpool.tile([128, 8, 512], BF16, tag="wtb")
                for i in range(4):
                    nc.sync.dma_start_transpose(
                        out=wtb[:, :, 128 * i : 128 * (i + 1)], in_=gts[i]
                    )
                wt8 = wtpool.tile([128, 8, 512], FP8, tag="wt")
                cd = nc.gpsimd.dma_start(out=wt8, in_=wtb)
                if g % 4:
                    cd.ins.queue = f"qPoolDynamic{g % 4}"

                # mm1: spectrum tiles [m(128), row(512)] via fp8 DoubleRow
                # (contraction 256 per matmul); P = Re^2 + Im^2 in bf16
                p_sb = ppool.tile([128, 5, 512], BF16, tag="p")
                for pair in range(5):
                    mm1 = mm1_ps_pool.tile([128, 1024], F32, tag="mm1")
                    for q in range(4):
                        nc.tensor.matmul(
                            mm1[:, 0:512],
                            cs1_sb[:, 2 * q : 2 * q + 2,
                                   128 * pair : 128 * (pair + 1)],
                            wt8[:, 2 * q : 2 * q + 2, :],
                            start=(q == 0),
                            stop=(q == 3),
                            perf_mode=mybir.MatmulPerfMode.DoubleRow,
                        )
                    if pair < 4:
                        for q in range(4):
                            nc.tensor.matmul(
                                mm1[:, 512:1024],
                                cs1_sb[:, 2 * q : 2 * q + 2,
                                       640 + 128 * pair : 640 + 128 * (pair + 1)],
                                wt8[:, 2 * q : 2 * q + 2, :],
                                start=(q == 0),
                                stop=(q == 3),
                                perf_mode=mybir.MatmulPerfMode.DoubleRow,
                            )
                        sq = sqpool.tile([128, 1024], BF16, tag="sq")
                        nc.scalar.square(sq, mm1)
                        nc.vector.tensor_add(
                            p_sb[:, pair, :], sq[:, 0:512], sq[:, 512:1024]
                        )
                    else:
                        # sin(m) = 0 for the whole pad tile: P = cos^2 only
                        nc.scalar.square(p_sb[:, pair, :], mm1[:, 0:512])

                # mm2: acov[row, 0:513] = P.T @ C2 (bf16), then expand by
                # symmetry: y[512+d] = acov[d], y[j] = acov[512-j]; scale by
                # 1/fmax^2 (fwd half on Scalar, reversed half on Vector)
                for rt in range(4):
                    mm2 = mm2_ps_pool.tile([128, 520], F32, tag="mm2")
                    for chunk in range(5):
                        nc.tensor.matmul(
                            mm2[:, 0:512],
                            p_sb[:, chunk, 128 * rt : 128 * (rt + 1)],
                            c2_sb[:, chunk, 0:512],
                            start=(chunk == 0),
                            stop=(chunk == 4),
                        )
                    # only d=512 is real output (y[0]); N=2 is the matmul
                    # minimum, cols 513 is junk
                    for chunk in range(5):
                        nc.tensor.matmul(
                            mm2[:, 512:514],
                            p_sb[:, chunk, 128 * rt : 128 * (rt + 1)],
                            c2_sb[:, chunk, 512:514],
                            start=(chunk == 0),
                            stop=(chunk == 4),
                        )
                    # single PSUM consumer: scale acov[0:514] into SBUF
                    # staging (frees the mm2 bank for the next rt fast); the
                    # forward y half DMAs straight from staging, the reversed
                    # half is an off-critical-path SBUF copy on Scalar.
                    hs = h0 + 16 * rt
                    dsty = y[b, hs : hs + 16].rearrange("h c n -> (h c) n")
                    ysbf = ypool.tile([128, 514], BF16, tag="yf")
                    if rt % 2 == 0:
                        nc.vector.tensor_scalar_mul(ysbf, mm2[:, 0:514],
                                                    inv2s[rt])
                    else:
                        nc.scalar.mul(ysbf, mm2[:, 0:514], inv2s[rt])
                    nc.sync.dma_start(out=dsty[:, 512:1024],
                                      in_=ysbf[:, 0:512])
                    ysbr = ypool.tile([128, 512], BF16, tag="yr")
                    rev = bass.AP(
                        tensor=ysbf.tensor,
                        offset=ysbf.offset + 512,
                        ap=[list(ysbf.ap[0]), [-1, 512]],
                    )
                    if rt % 2 == 0:
                        nc.scalar.copy(out=ysbr, in_=rev)
                    else:
                        nc.vector.tensor_copy(out=ysbr, in_=rev)
                    nc.sync.dma_start(out=dsty[:, 0:512], in_=ysbr)
    return nc


def get_program():
    if "nc" not in _prog_cache:
        _prog_cache["nc"] = build_program()
        _prog_cache["consts"] = make_constants()
    return _prog_cache["nc"], _prog_cache["consts"]


def kernel(X, kernel_size=None, out_channels=None, _trace=False):
    X = np.asarray(X, dtype=np.float32)
    assert X.shape == (B_FULL, C_DIM, L_DIM)
    Xb = np.ascontiguousarray(X.astype(NP_BF16))
    nc, consts = get_program()
    in_maps = []
    for c in range(N_CORES):
        m = {"x": Xb[c * B_LOC : (c + 1) * B_LOC]}
        m.update(consts)
        in_maps.append(m)
    res = run_bass_kernel_spmd(
        nc, in_maps, core_ids=list(range(N_CORES)), trace=_trace
    )
    out = np.concatenate(
        [r["y"].astype(np.float32) for r in res.results], axis=0
    )
    if _trace:
        return out, res
    return out


# revision 36
# speedup vs baseline: 1.5297x; 1.5297x over previous
"""Trainium2 Bass kernel for LocalPatternFilter.

Reference computation (per (b, h, c) row of length K=1024):
  1. gather window  g = X[b, c, pos[h] : pos[h]+K]
  2. fmax = max|g|;  w = g * hann / fmax
  3. acov = ifftshift(irfft(|rfft(w)|^2))   (= circular autocorrelation)

Implemented as dense DFT matmuls on the tensor engine:
  - X converted to bf16 on host; windows gathered bf16 (indirect DMA).
  - rfft via fp8(e4m3) DoubleRow matmuls (contraction 256/instr, 2 fp8
    weights per PE cell): cos/sin matrices with hann folded in, quantized
    to fp8; window data cast bf16->fp8 during the PSUM->SBUF copy after
    the (bf16) PE transposes.
  - power spectrum P = Re^2 + Im^2 in bf16 (squares split across the
    Scalar and Vector engines).
  - irfft + ifftshift + output symmetry via a bf16 (640 x 520) cos matrix;
    1/fmax^2 folded into the PSUM->SBUF scale copies (Scalar fwd half,
    Vector reversed half). Output y in bf16, upcast to f32 on host.

Sharding: data-parallel over batch, 2 batches per core on 8 cores.
"""

import json

import ml_dtypes
import numpy as np

import concourse.bass as bass
import concourse.bass2jax as bass2jax
import concourse.bass_utils as bass_utils
import concourse.tile as tile
from concourse import mybir
from concourse.bass_utils import run_bass_kernel_spmd

# ---------------------------------------------------------------------------
# The walrus build in this container accepts at most ONE sync-wait command per
# TPB instruction ("Too many sync wait commands" in setupSyncWait), while Tile
# emits several (multi-queue DMA deps, the kernel-tail drain). Legalize the
# serialized BIR before compiling: hoist excess waits onto preceding
# same-engine wait-only EventSemaphore instructions. Engines execute their
# instruction streams in order, so this is semantically identical.
# ---------------------------------------------------------------------------
_MAX_WAITS = 1


def _legalize_bir(bir_bytes):
    m = json.loads(bir_bytes)
    counter = [0]

    def fix_block(blk):
        insts = blk.get("instructions")
        if not isinstance(insts, list):
            return
        out = []
        for inst in insts:
            si = inst.get("sync_info") or {}
            waits = si.get("on_wait") or []
            if isinstance(inst.get("opcode"), str) and len(waits) > _MAX_WAITS:
                keep = waits[-_MAX_WAITS:]
                for w in waits[:-_MAX_WAITS]:
                    counter[0] += 1
                    out.append(
                        {
                            "debug": inst.get("debug", 0),
                            "engine": inst["engine"],
                            "ins": [],
                            "name": f"LGW-{counter[0]}-{inst['name']}",
                            "opcode": "EventSemaphore",
                            "outs": [],
                            "sync_info": {"on_update": [], "on_wait": [w]},
                        }
                    )
                si = dict(si)
                si["on_wait"] = keep
                inst = dict(inst)
                inst["sync_info"] = si
            out.append(inst)
        blk["instructions"] = out

    def walk(o):
        if isinstance(o, dict):
            if "instructions" in o:
                fix_block(o)
            for v in o.values():
                walk(v)
        elif isinstance(o, list):
            for v in o:
                walk(v)

    walk(m)
    return json.dumps(m).encode()


_orig_compile_bir_kernel = bass_utils.compile_bir_kernel


def _legalizing_compile_bir_kernel(bir_json, *args, **kwargs):
    if isinstance(bir_json, str):
        bir_json = bir_json.encode()
    return _orig_compile_bir_kernel(_legalize_bir(bir_json), *args, **kwargs)


if bass_utils.compile_bir_kernel is not _legalizing_compile_bir_kernel:
    bass_utils.compile_bir_kernel = _legalizing_compile_bir_kernel
    bass2jax.compile_bir_kernel = _legalizing_compile_bir_kernel

# positions = int32(jnp.linspace(0, L-2K-1, H)) for L=131072, K=1024, H=128
# (verified identical to the jax reference's values)
POS = [
    0, 1015, 2031, 3047, 4063, 5079, 6095, 7111, 8127, 9143, 10159, 11175,
    12191, 13207, 14223, 15238, 16254, 17270, 18286, 19302, 20318, 21334,
    22350, 23366, 24382, 25398, 26414, 27430, 28446, 29461, 30477, 31493,
    32509, 33525, 34541, 35557, 36573, 37589, 38605, 39621, 40637, 41653,
    42669, 43684, 44700, 45716, 46732, 47748, 48764, 49780, 50796, 51812,
    52828, 53844, 54860, 55876, 56892, 57907, 58923, 59939, 60955, 61971,
    62987, 64003, 65019, 66035, 67051, 68067, 69083, 70099, 71115, 72130,
    73146, 74162, 75178, 76194, 77210, 78226, 79242, 80258, 81274, 82290,
    83306, 84322, 85338, 86353, 87369, 88385, 89401, 90417, 91433, 92449,
    93465, 94481, 95497, 96513, 97529, 98545, 99561, 100576, 101592, 102608,
    103624, 104640, 105656, 106672, 107688, 108704, 109720, 110736, 111752,
    112768, 113784, 114799, 115815, 116831, 117847, 118863, 119879, 120895,
    121911, 122927, 123943, 124959, 125975, 126991, 128007, 129023,
]

N_CORES = 8
B_FULL, C_DIM, L_DIM = 16, 8, 131072
K_DIM, H_DIM = 1024, 128
B_LOC = B_FULL // N_CORES          # batches per core
M_PAD = 640                        # rfft bins 0..512, padded to 5*128
CS_W = 1152                        # 640 windowed-cos cols + 512 windowed-sin
BF16 = mybir.dt.bfloat16
FP8 = mybir.dt.float8e4
F32 = mybir.dt.float32

NP_BF16 = ml_dtypes.bfloat16
NP_FP8 = ml_dtypes.float8_e4m3

_prog_cache = {}


def make_constants():
    K = K_DIM
    k = np.arange(K)[:, None].astype(np.float64)
    hann = 0.5 * (1.0 - np.cos(2.0 * np.pi * np.arange(K) / K))[:, None]
    m = np.arange(M_PAD)[None, :].astype(np.float64)
    cs_cos = hann * np.cos(2 * np.pi * k * m / K)
    cs_cos[:, 513:] = 0.0
    m2 = np.arange(512)[None, :].astype(np.float64)
    cs_sin = hann * np.sin(2 * np.pi * k * m2 / K)
    # cs1[t, p, :] = [win*cos m=0..639 | win*sin m=0..511] for k = 128 t + p
    cs1 = np.concatenate([cs_cos, cs_sin], axis=1)  # (1024, 1152)
    cs1 = cs1.reshape(8, 128, CS_W).astype(NP_FP8)

    # irfft weights for acov[d], d = 0..512 only; the full output row is
    # y[512+d] = acov[d], y[j] = acov[512-j] (acov is even), written via a
    # reversed-stride copy.
    wt = np.ones(M_PAD)
    wt[1:512] = 2.0
    wt[512] = 1.0
    wt[513:] = 0.0
    mm = np.arange(M_PAD)[:, None].astype(np.float64)
    d = np.arange(513)[None, :].astype(np.float64)
    c2 = wt[:, None] * np.cos(2 * np.pi * mm * d / K) / K
    c2 = np.concatenate([c2, np.zeros((M_PAD, 7))], axis=1)  # pad to 520
    c2 = c2.reshape(5, 128, 520).astype(NP_BF16)

    ident = np.eye(128, dtype=np.float32).astype(NP_BF16)

    # gather start offsets (elements into flat [B_LOC*C*L] x) per tile row:
    # tile j = g*4 + i; row p = hh*8 + c; h = (g%2)*64 + 16*i + hh
    gidx = np.zeros((16, 128), dtype=np.int32)
    for g in range(2 * B_LOC):
        b = g // 2
        h0 = (g % 2) * 64
        for i in range(4):
            for hh in range(16):
                h = h0 + 16 * i + hh
                for c in range(C_DIM):
                    gidx[g * 4 + i, hh * 8 + c] = (
                        b * C_DIM * L_DIM + c * L_DIM + POS[h]
                    )
    return {"cs1": cs1, "c2": c2, "ident": ident, "gidx": gidx}


def build_program():
    nc = bass.Bass("TRN2", target_bir_lowering=False, debug=False,
                   num_swdge_queues=4)
    x = nc.dram_tensor("x", [B_LOC, C_DIM, L_DIM], BF16, kind="ExternalInput").ap()
    cs1 = nc.dram_tensor("cs1", [8, 128, CS_W], FP8, kind="ExternalInput").ap()
    c2 = nc.dram_tensor("c2", [5, 128, 520], BF16, kind="ExternalInput").ap()
    ident = nc.dram_tensor("ident", [128, 128], BF16, kind="ExternalInput").ap()
    gidx = nc.dram_tensor("gidx", [16, 128], mybir.dt.int32, kind="ExternalInput").ap()
    y = nc.dram_tensor(
        "y", [B_LOC, H_DIM, C_DIM, K_DIM], BF16, kind="ExternalOutput"
    ).ap()

    with tile.TileContext(nc) as tc:
        with (
            tc.tile_pool(name="singles", bufs=1) as singles,
            tc.tile_pool(name="gather", bufs=8) as gpool,
            tc.tile_pool(name="wt", bufs=2) as wtpool,
            tc.tile_pool(name="pp", bufs=2) as ppool,
            tc.tile_pool(name="yy", bufs=4) as ypool,
            tc.tile_pool(name="small", bufs=16) as smallpool,
            tc.tile_pool(name="sq", bufs=3) as sqpool,
            tc.tile_pool(name="tp_ps", bufs=2, space="PSUM") as tp_ps_pool,
            tc.tile_pool(name="mm1_ps", bufs=2, space="PSUM") as mm1_ps_pool,
            tc.tile_pool(name="mm2_ps", bufs=1, space="PSUM") as mm2_ps_pool,
        ):
            # gidx first so gathers can start immediately; DFT matrices
            # stream in as single batched DMAs behind it
            gidx_sb = singles.tile([128, 16], mybir.dt.int32)
            nc.sync.dma_start(out=gidx_sb, in_=gidx.rearrange("t p -> p t"))
            x_flat = x.rearrange("b c l -> (b c) l")
            cs1_sb = singles.tile([128, 8, CS_W], FP8)
            nc.sync.dma_start(out=cs1_sb, in_=cs1.rearrange("t p m -> p t m"))
            c2_sb = singles.tile([128, 5, 520], BF16)
            nc.sync.dma_start(out=c2_sb, in_=c2.rearrange("t p n -> p t n"))
            # ident loads last: its 256B-descriptor DMA is slow and nothing
            # needs it until the first transposes (~20us in)
            id_sb = singles.tile([128, 128], BF16)
            nc.sync.dma_start(out=id_sb, in_=ident)

            # HAM warmup: the PE clock-gate needs ~3.4us of sustained matmul
            # activity to unthrottle 1.2 -> 2.4 GHz. Burn junk DoubleRow
            # matmuls on cs1 while the first gathers are still in flight so
            # the real pipeline starts at full clock.
            warm = mm1_ps_pool.tile([128, 1024], F32, tag="mm1")
            for w in range(14):
                nc.tensor.matmul(
                    warm[:, 0:512],
                    cs1_sb[:, 0:2, 0:128],
                    cs1_sb[:, 0:2, 0:512],
                    start=(w == 0),
                    stop=(w == 13),
                    perf_mode=mybir.MatmulPerfMode.DoubleRow,
                )

            # 4 groups of 512 rows; row = b*1024 + h*8 + c
            for g in range(2 * B_LOC):
                b = g // 2
                h0 = (g % 2) * 64
                gts = []
                inv2s = []
                for i in range(4):
                    gt = gpool.tile([128, K_DIM], BF16, tag="gt")
                    j = g * 4 + i
                    gd = nc.gpsimd.indirect_dma_start(
                        out=gt[:],
                        out_offset=None,
                        in_=x_flat,
                        in_offset=bass.IndirectOffsetOnAxis(
                            ap=gidx_sb[:, j : j + 1], axis=1
                        ),
                    )
                    qi = j % 4
                    if qi:
                        gd.ins.queue = f"qPoolDynamic{qi}"  # spread SWDGE queues
                    fm = smallpool.tile([128, 1], BF16, tag="fm")
                    nc.vector.reduce_max(
                        out=fm, in_=gt,
                        axis=mybir.AxisListType.X,
                        apply_absolute_value=True,
                    )
                    inv = smallpool.tile([128, 1], F32, tag="inv")
                    nc.vector.reciprocal(out=inv, in_=fm)
                    inv2 = smallpool.tile([128, 1], F32, tag="inv2")
                    nc.vector.tensor_mul(inv2, inv, inv)
                    gts.append(gt)
                    inv2s.append(inv2)

                # transpose gathered rows (bf16) to [k, row] layout on the PE;
                # the PSUM->SBUF copy also casts bf16 -> fp8 for the DoubleRow
                # matmuls. Copies alternate Vector/Scalar to balance load.
                wt8 = wtpool.tile([128, 8, 512], FP8, tag="wt")
                for t in range(8):
                    tp = tp_ps_pool.tile([128, 512], BF16, tag="tp")
                    for i in range(4):
                        nc.tensor.transpose(
                            tp[:, 128 * i : 128 * (i + 1)],
                            gts[i][:, 128 * t : 128 * (t + 1)],
                            id_sb,
                        )
                    if t % 2 == 0:
                        nc.vector.tensor_copy(out=wt8[:, t, :], in_=tp)
                    else:
                        nc.scalar.copy(out=wt8[:, t, :], in_=tp)

                # mm1: spectrum tiles [m(128), row(512)] via fp8 DoubleRow
                # (contraction 256 per matmul); P = Re^2 + Im^2 in bf16
                p_sb = ppool.tile([128, 5, 512], BF16, tag="p")
                for pair in range(5):
                    mm1 = mm1_ps_pool.tile([128, 1024], F32, tag="mm1")
                    for q in range(4):
                        nc.tensor.matmul(
                            mm1[:, 0:512],
                            cs1_sb[:, 2 * q : 2 * q + 2,
                                   128 * pair : 128 * (pair + 1)],
                            wt8[:, 2 * q : 2 * q + 2, :],
                            start=(q == 0),
                            stop=(q == 3),
                            perf_mode=mybir.MatmulPerfMode.DoubleRow,
                        )
                    if pair < 4:
                        for q in range(4):
                            nc.tensor.matmul(
                                mm1[:, 512:1024],
                                cs1_sb[:, 2 * q : 2 * q + 2,
                                       640 + 128 * pair : 640 + 128 * (pair + 1)],
                                wt8[:, 2 * q : 2 * q + 2, :],
                                start=(q == 0),
                                stop=(q == 3),
                                perf_mode=mybir.MatmulPerfMode.DoubleRow,
                            )
                        sq = sqpool.tile([128, 1024], BF16, tag="sq")
                        nc.scalar.square(sq, mm1)
                        nc.vector.tensor_add(
                            p_sb[:, pair, :], sq[:, 0:512], sq[:, 512:1024]
                        )
                    else:
                        # sin(m) = 0 for the whole pad tile: P = cos^2 only
                        nc.scalar.square(p_sb[:, pair, :], mm1[:, 0:512])

                # mm2: acov[row, 0:513] = P.T @ C2 (bf16), then expand by
                # symmetry: y[512+d] = acov[d], y[j] = acov[512-j]; scale by
                # 1/fmax^2 (fwd half on Scalar, reversed half on Vector)
                for rt in range(4):
                    mm2 = mm2_ps_pool.tile([128, 520], F32, tag="mm2")
                    for chunk in range(5):
                        nc.tensor.matmul(
                            mm2[:, 0:512],
                            p_sb[:, chunk, 128 * rt : 128 * (rt + 1)],
                            c2_sb[:, chunk, 0:512],
                            start=(chunk == 0),
                            stop=(chunk == 4),
                        )
                    # only d=512 is real output (y[0]); N=2 is the matmul
                    # minimum, cols 513 is junk
                    for chunk in range(5):
                        nc.tensor.matmul(
                            mm2[:, 512:514],
                            p_sb[:, chunk, 128 * rt : 128 * (rt + 1)],
                            c2_sb[:, chunk, 512:514],
                            start=(chunk == 0),
                            stop=(chunk == 4),
                        )
                    # single PSUM consumer: scale acov[0:514] into SBUF
                    # staging (frees the mm2 bank for the next rt fast); the
                    # forward y half DMAs straight from staging, the reversed
                    # half is an off-critical-path SBUF copy on Scalar.
                    hs = h0 + 16 * rt
                    dsty = y[b, hs : hs + 16].rearrange("h c n -> (h c) n")
                    ysbf = ypool.tile([128, 514], BF16, tag="yf")
                    if rt % 2 == 0:
                        nc.vector.tensor_scalar_mul(ysbf, mm2[:, 0:514],
                                                    inv2s[rt])
                    else:
                        nc.scalar.mul(ysbf, mm2[:, 0:514], inv2s[rt])
                    nc.sync.dma_start(out=dsty[:, 512:1024],
                                      in_=ysbf[:, 0:512])
                    ysbr = ypool.tile([128, 512], BF16, tag="yr")
                    rev = bass.AP(
                        tensor=ysbf.tensor,
                        offset=ysbf.offset + 512,
                        ap=[list(ysbf.ap[0]), [-1, 512]],
                    )
                    if rt % 2 == 0:
                        nc.scalar.copy(out=ysbr, in_=rev)
                    else:
                        nc.vector.tensor_copy(out=ysbr, in_=rev)
                    nc.sync.dma_start(out=dsty[:, 0:512], in_=ysbr)
    return nc


def get_program():
    if "nc" not in _prog_cache:
        _prog_cache["nc"] = build_program()
        _prog_cache["consts"] = make_constants()
    return _prog_cache["nc"], _prog_cache["consts"]


def kernel(X, kernel_size=None, out_channels=None, _trace=False):
    X = np.asarray(X, dtype=np.float32)
    assert X.shape == (B_FULL, C_DIM, L_DIM)
    Xb = np.ascontiguousarray(X.astype(NP_BF16))
    nc, consts = get_program()
    in_maps = []
    for c in range(N_CORES):
        m = {"x": Xb[c * B_LOC : (c + 1) * B_LOC]}
        m.update(consts)
        in_maps.append(m)
    res = run_bass_kernel_spmd(
        nc, in_maps, core_ids=list(range(N_CORES)), trace=_trace
    )
    out = np.concatenate(
        [r["y"].astype(np.float32) for r in res.results], axis=0
    )
    if _trace:
        return out, res
    return out


# revision 42
# speedup vs baseline: 1.6142x; 1.0553x over previous
"""Trainium2 Bass kernel for LocalPatternFilter.

Reference computation (per (b, h, c) row of length K=1024):
  1. gather window  g = X[b, c, pos[h] : pos[h]+K]
  2. fmax = max|g|;  w = g * hann / fmax
  3. acov = ifftshift(irfft(|rfft(w)|^2))   (= circular autocorrelation)

Implemented as dense DFT matmuls on the tensor engine:
  - X converted to bf16 on host; windows gathered bf16 (indirect DMA).
  - rfft via fp8(e4m3) DoubleRow matmuls (contraction 256/instr, 2 fp8
    weights per PE cell): cos/sin matrices with hann folded in, quantized
    to fp8; window data cast bf16->fp8 during the PSUM->SBUF copy after
    the (bf16) PE transposes.
  - power spectrum P = Re^2 + Im^2 in bf16 (squares split across the
    Scalar and Vector engines).
  - irfft + ifftshift + output symmetry via a bf16 (640 x 520) cos matrix;
    1/fmax^2 folded into the PSUM->SBUF scale copies (Scalar fwd half,
    Vector reversed half). Output y in bf16, upcast to f32 on host.

Sharding: data-parallel over batch, 2 batches per core on 8 cores.
"""

import json

import ml_dtypes
import numpy as np

import concourse.bass as bass
import concourse.bass2jax as bass2jax
import concourse.bass_utils as bass_utils
import concourse.tile as tile
from concourse import mybir
from concourse.bass_utils import run_bass_kernel_spmd

# ---------------------------------------------------------------------------
# The walrus build in this container accepts at most ONE sync-wait command per
# TPB instruction ("Too many sync wait commands" in setupSyncWait), while Tile
# emits several (multi-queue DMA deps, the kernel-tail drain). Legalize the
# serialized BIR before compiling: hoist excess waits onto preceding
# same-engine wait-only EventSemaphore instructions. Engines execute their
# instruction streams in order, so this is semantically identical.
# ---------------------------------------------------------------------------
_MAX_WAITS = 1


def _legalize_bir(bir_bytes):
    m = json.loads(bir_bytes)
    counter = [0]

    def fix_block(blk):
        insts = blk.get("instructions")
        if not isinstance(insts, list):
            return
        out = []
        for inst in insts:
            si = inst.get("sync_info") or {}
            waits = si.get("on_wait") or []
            if isinstance(inst.get("opcode"), str) and len(waits) > _MAX_WAITS:
                keep = waits[-_MAX_WAITS:]
                for w in waits[:-_MAX_WAITS]:
                    counter[0] += 1
                    out.append(
                        {
                            "debug": inst.get("debug", 0),
                            "engine": inst["engine"],
                            "ins": [],
                            "name": f"LGW-{counter[0]}-{inst['name']}",
                            "opcode": "EventSemaphore",
                            "outs": [],
                            "sync_info": {"on_update": [], "on_wait": [w]},
                        }
                    )
                si = dict(si)
                si["on_wait"] = keep
                inst = dict(inst)
                inst["sync_info"] = si
            out.append(inst)
        blk["instructions"] = out

    def walk(o):
        if isinstance(o, dict):
            if "instructions" in o:
                fix_block(o)
            for v in o.values():
                walk(v)
        elif isinstance(o, list):
            for v in o:
                walk(v)

    walk(m)
    return json.dumps(m).encode()


_orig_compile_bir_kernel = bass_utils.compile_bir_kernel


def _legalizing_compile_bir_kernel(bir_json, *args, **kwargs):
    if isinstance(bir_json, str):
        bir_json = bir_json.encode()
    return _orig_compile_bir_kernel(_legalize_bir(bir_json), *args, **kwargs)


if bass_utils.compile_bir_kernel is not _legalizing_compile_bir_kernel:
    bass_utils.compile_bir_kernel = _legalizing_compile_bir_kernel
    bass2jax.compile_bir_kernel = _legalizing_compile_bir_kernel

# positions = int32(jnp.linspace(0, L-2K-1, H)) for L=131072, K=1024, H=128
# (verified identical to the jax reference's values)
POS = [
    0, 1015, 2031, 3047, 4063, 5079, 6095, 7111, 8127, 9143, 10159, 11175,
    12191, 13207, 14223, 15238, 16254, 17270, 18286, 19302, 20318, 21334,
    22350, 23366, 24382, 25398, 26414, 27430, 28446, 29461, 30477, 31493,
    32509, 33525, 34541, 35557, 36573, 37589, 38605, 39621, 40637, 41653,
    42669, 43684, 44700, 45716, 46732, 47748, 48764, 49780, 50796, 51812,
    52828, 53844, 54860, 55876, 56892, 57907, 58923, 59939, 60955, 61971,
    62987, 64003, 65019, 66035, 67051, 68067, 69083, 70099, 71115, 72130,
    73146, 74162, 75178, 76194, 77210, 78226, 79242, 80258, 81274, 82290,
    83306, 84322, 85338, 86353, 87369, 88385, 89401, 90417, 91433, 92449,
    93465, 94481, 95497, 96513, 97529, 98545, 99561, 100576, 101592, 102608,
    103624, 104640, 105656, 106672, 107688, 108704, 109720, 110736, 111752,
    112768, 113784, 114799, 115815, 116831, 117847, 118863, 119879, 120895,
    121911, 122927, 123943, 124959, 125975, 126991, 128007, 129023,
]

N_CORES = 8
B_FULL, C_DIM, L_DIM = 16, 8, 131072
K_DIM, H_DIM = 1024, 128
B_LOC = B_FULL // N_CORES          # batches per core
M_BINS = 512                       # rfft bins 0..511 (Nyquist bin dropped:
                                   # its contribution is ~1e-3 of absmax)
CS_W = 1024                        # 512 windowed-cos cols + 512 windowed-sin
BF16 = mybir.dt.bfloat16
FP8 = mybir.dt.float8e4
F32 = mybir.dt.float32

NP_BF16 = ml_dtypes.bfloat16
NP_FP8 = ml_dtypes.float8_e4m3

_prog_cache = {}


def make_constants():
    K = K_DIM
    k = np.arange(K)[:, None].astype(np.float64)
    hann = 0.5 * (1.0 - np.cos(2.0 * np.pi * np.arange(K) / K))[:, None]
    m = np.arange(M_BINS)[None, :].astype(np.float64)
    cs_cos = hann * np.cos(2 * np.pi * k * m / K)
    cs_sin = hann * np.sin(2 * np.pi * k * m / K)
    # cs1[t, p, :] = [win*cos m=0..511 | win*sin m=0..511] for k = 128 t + p
    cs1 = np.concatenate([cs_cos, cs_sin], axis=1)  # (1024, 1024)
    cs1 = cs1.reshape(8, 128, CS_W).astype(NP_FP8)

    # irfft weights for acov[d], d = 0..513 computed (512+ junk); the full
    # output row is y[512+d] = acov[d], y[j] = acov[512-j] (acov is even),
    # written via a reversed-stride copy.
    wt = np.ones(M_BINS)
    wt[1:512] = 2.0
    mm = np.arange(M_BINS)[:, None].astype(np.float64)
    d = np.arange(513)[None, :].astype(np.float64)
    c2 = wt[:, None] * np.cos(2 * np.pi * mm * d / K) / K
    c2 = np.concatenate([c2, np.zeros((M_BINS, 7))], axis=1)  # pad to 520
    c2 = c2.reshape(4, 128, 520).astype(NP_BF16)

    ident = np.eye(128, dtype=np.float32).astype(NP_BF16)

    # gather start offsets (elements into flat [B_LOC*C*L] x) per tile row:
    # tile j = g*4 + i; row p = hh*8 + c; h = (g%2)*64 + 16*i + hh
    gidx = np.zeros((16, 128), dtype=np.int32)
    for g in range(2 * B_LOC):
        b = g // 2
        h0 = (g % 2) * 64
        for i in range(4):
            for hh in range(16):
                h = h0 + 16 * i + hh
                for c in range(C_DIM):
                    gidx[g * 4 + i, hh * 8 + c] = (
                        b * C_DIM * L_DIM + c * L_DIM + POS[h]
                    )
    return {"cs1": cs1, "c2": c2, "ident": ident, "gidx": gidx}


def build_program():
    nc = bass.Bass("TRN2", target_bir_lowering=False, debug=False,
                   num_swdge_queues=4)
    x = nc.dram_tensor("x", [B_LOC, C_DIM, L_DIM], BF16, kind="ExternalInput").ap()
    cs1 = nc.dram_tensor("cs1", [8, 128, CS_W], FP8, kind="ExternalInput").ap()
    c2 = nc.dram_tensor("c2", [4, 128, 520], BF16, kind="ExternalInput").ap()
    ident = nc.dram_tensor("ident", [128, 128], BF16, kind="ExternalInput").ap()
    gidx = nc.dram_tensor("gidx", [16, 128], mybir.dt.int32, kind="ExternalInput").ap()
    y = nc.dram_tensor(
        "y", [B_LOC, H_DIM, C_DIM, K_DIM], BF16, kind="ExternalOutput"
    ).ap()

    with tile.TileContext(nc) as tc:
        with (
            tc.tile_pool(name="singles", bufs=1) as singles,
            tc.tile_pool(name="gather", bufs=8) as gpool,
            tc.tile_pool(name="wt", bufs=2) as wtpool,
            tc.tile_pool(name="pp", bufs=2) as ppool,
            tc.tile_pool(name="yy", bufs=4) as ypool,
            tc.tile_pool(name="small", bufs=16) as smallpool,
            tc.tile_pool(name="sq", bufs=3) as sqpool,
            tc.tile_pool(name="tp_ps", bufs=2, space="PSUM") as tp_ps_pool,
            tc.tile_pool(name="mm1_ps", bufs=2, space="PSUM") as mm1_ps_pool,
            tc.tile_pool(name="mm2_ps", bufs=1, space="PSUM") as mm2_ps_pool,
        ):
            # gidx first so gathers can start immediately; DFT matrices
            # stream in as single batched DMAs behind it
            gidx_sb = singles.tile([128, 16], mybir.dt.int32)
            nc.sync.dma_start(out=gidx_sb, in_=gidx.rearrange("t p -> p t"))
            x_flat = x.rearrange("b c l -> (b c) l")
            cs1_sb = singles.tile([128, 8, CS_W], FP8)
            nc.sync.dma_start(out=cs1_sb, in_=cs1.rearrange("t p m -> p t m"))
            c2_sb = singles.tile([128, 4, 520], BF16)
            nc.sync.dma_start(out=c2_sb, in_=c2.rearrange("t p n -> p t n"))
            # ident loads last: its 256B-descriptor DMA is slow and nothing
            # needs it until the first transposes (~20us in)
            id_sb = singles.tile([128, 128], BF16)
            nc.sync.dma_start(out=id_sb, in_=ident)

            # HAM warmup: the PE clock-gate needs ~3.4us of sustained matmul
            # activity to unthrottle 1.2 -> 2.4 GHz. Burn junk DoubleRow
            # matmuls on cs1 while the first gathers are still in flight so
            # the real pipeline starts at full clock.
            warm = mm1_ps_pool.tile([128, 1024], F32, tag="mm1")
            for w in range(14):
                nc.tensor.matmul(
                    warm[:, 0:512],
                    cs1_sb[:, 0:2, 0:128],
                    cs1_sb[:, 0:2, 0:512],
                    start=(w == 0),
                    stop=(w == 13),
                    perf_mode=mybir.MatmulPerfMode.DoubleRow,
                )

            # 4 groups of 512 rows; row = b*1024 + h*8 + c
            for g in range(2 * B_LOC):
                b = g // 2
                h0 = (g % 2) * 64
                gts = []
                inv2s = []
                for i in range(4):
                    gt = gpool.tile([128, K_DIM], BF16, tag="gt")
                    j = g * 4 + i
                    gd = nc.gpsimd.indirect_dma_start(
                        out=gt[:],
                        out_offset=None,
                        in_=x_flat,
                        in_offset=bass.IndirectOffsetOnAxis(
                            ap=gidx_sb[:, j : j + 1], axis=1
                        ),
                    )
                    qi = j % 4
                    if qi:
                        gd.ins.queue = f"qPoolDynamic{qi}"  # spread SWDGE queues
                    fm = smallpool.tile([128, 1], BF16, tag="fm")
                    nc.vector.reduce_max(
                        out=fm, in_=gt,
                        axis=mybir.AxisListType.X,
                        apply_absolute_value=True,
                    )
                    inv = smallpool.tile([128, 1], F32, tag="inv")
                    nc.vector.reciprocal(out=inv, in_=fm)
                    inv2 = smallpool.tile([128, 1], F32, tag="inv2")
                    nc.vector.tensor_mul(inv2, inv, inv)
                    gts.append(gt)
                    inv2s.append(inv2)

                # transpose gathered rows (bf16) to [k, row] layout on the PE;
                # the PSUM->SBUF copy also casts bf16 -> fp8 for the DoubleRow
                # matmuls. Copies alternate Vector/Scalar to balance load.
                wt8 = wtpool.tile([128, 8, 512], FP8, tag="wt")
                for t in range(8):
                    tp = tp_ps_pool.tile([128, 512], BF16, tag="tp")
                    for i in range(4):
                        nc.tensor.transpose(
                            tp[:, 128 * i : 128 * (i + 1)],
                            gts[i][:, 128 * t : 128 * (t + 1)],
                            id_sb,
                        )
                    if t % 2 == 0:
                        nc.vector.tensor_copy(out=wt8[:, t, :], in_=tp)
                    else:
                        nc.scalar.copy(out=wt8[:, t, :], in_=tp)

                # mm1: spectrum tiles [m(128), row(512)] via fp8 DoubleRow
                # (contraction 256 per matmul); P = Re^2 + Im^2 in bf16
                p_sb = ppool.tile([128, 4, 512], BF16, tag="p")
                for pair in range(4):
                    mm1 = mm1_ps_pool.tile([128, 1024], F32, tag="mm1")
                    for q in range(4):
                        nc.tensor.matmul(
                            mm1[:, 0:512],
                            cs1_sb[:, 2 * q : 2 * q + 2,
                                   128 * pair : 128 * (pair + 1)],
                            wt8[:, 2 * q : 2 * q + 2, :],
                            start=(q == 0),
                            stop=(q == 3),
                            perf_mode=mybir.MatmulPerfMode.DoubleRow,
                        )
                    for q in range(4):
                        nc.tensor.matmul(
                            mm1[:, 512:1024],
                            cs1_sb[:, 2 * q : 2 * q + 2,
                                   512 + 128 * pair : 512 + 128 * (pair + 1)],
                            wt8[:, 2 * q : 2 * q + 2, :],
                            start=(q == 0),
                            stop=(q == 3),
                            perf_mode=mybir.MatmulPerfMode.DoubleRow,
                        )
                    sq = sqpool.tile([128, 1024], BF16, tag="sq")
                    nc.scalar.square(sq, mm1)
                    nc.vector.tensor_add(
                        p_sb[:, pair, :], sq[:, 0:512], sq[:, 512:1024]
                    )

                # mm2: acov[row, 0:513] = P.T @ C2 (bf16), then expand by
                # symmetry: y[512+d] = acov[d], y[j] = acov[512-j]; scale by
                # 1/fmax^2 (fwd half on Scalar, reversed half on Vector)
                for rt in range(4):
                    mm2 = mm2_ps_pool.tile([128, 520], F32, tag="mm2")
                    for chunk in range(4):
                        nc.tensor.matmul(
                            mm2[:, 0:512],
                            p_sb[:, chunk, 128 * rt : 128 * (rt + 1)],
                            c2_sb[:, chunk, 0:512],
                            start=(chunk == 0),
                            stop=(chunk == 3),
                        )
                    # only d=512 is real output (y[0]); N=2 is the matmul
                    # minimum, cols 513 is junk
                    for chunk in range(4):
                        nc.tensor.matmul(
                            mm2[:, 512:514],
                            p_sb[:, chunk, 128 * rt : 128 * (rt + 1)],
                            c2_sb[:, chunk, 512:514],
                            start=(chunk == 0),
                            stop=(chunk == 3),
                        )
                    # single PSUM consumer: scale acov[0:514] into SBUF
                    # staging (frees the mm2 bank for the next rt fast); the
                    # forward y half DMAs straight from staging, the reversed
                    # half is an off-critical-path SBUF copy on Scalar.
                    hs = h0 + 16 * rt
                    dsty = y[b, hs : hs + 16].rearrange("h c n -> (h c) n")
                    ysbf = ypool.tile([128, 514], BF16, tag="yf")
                    if rt % 2 == 0:
                        nc.vector.tensor_scalar_mul(ysbf, mm2[:, 0:514],
                                                    inv2s[rt])
                    else:
                        nc.scalar.mul(ysbf, mm2[:, 0:514], inv2s[rt])
                    nc.sync.dma_start(out=dsty[:, 512:1024],
                                      in_=ysbf[:, 0:512])
                    ysbr = ypool.tile([128, 512], BF16, tag="yr")
                    rev = bass.AP(
                        tensor=ysbf.tensor,
                        offset=ysbf.offset + 512,
                        ap=[list(ysbf.ap[0]), [-1, 512]],
                    )
                    if rt % 2 == 0:
                        nc.scalar.copy(out=ysbr, in_=rev)
                    else:
                        nc.vector.tensor_copy(out=ysbr, in_=rev)
                    nc.sync.dma_start(out=dsty[:, 0:512], in_=ysbr)
    return nc


def get_program():
    if "nc" not in _prog_cache:
        _prog_cache["nc"] = build_program()
        _prog_cache["consts"] = make_constants()
    return _prog_cache["nc"], _prog_cache["consts"]


def kernel(X, kernel_size=None, out_channels=None, _trace=False):
    X = np.asarray(X, dtype=np.float32)
    assert X.shape == (B_FULL, C_DIM, L_DIM)
    Xb = np.ascontiguousarray(X.astype(NP_BF16))
    nc, consts = get_program()
    in_maps = []
    for c in range(N_CORES):
        m = {"x": Xb[c * B_LOC : (c + 1) * B_LOC]}
        m.update(consts)
        in_maps.append(m)
    res = run_bass_kernel_spmd(
        nc, in_maps, core_ids=list(range(N_CORES)), trace=_trace
    )
    out = np.concatenate(
        [r["y"].astype(np.float32) for r in res.results], axis=0
    )
    if _trace:
        return out, res
    return out
